# revision 26
# baseline (speedup 1.0000x reference)
"""Trainium2 Bass kernel for single-step AttnDecoderRNN (batch=1 decode).

Strategy (8-way tensor parallel, one NEFF, SPMD):
  - embedding gather happens on host (only the one needed row is shipped)
  - attention (attn_W, encoder_outputs) replicated: every core computes the
    full 512 attn weights and the full attended context (cheap: ~9MB)
  - attn_combine row-sharded over H  -> x_c [256]     -> AllGather -> x [2048]
  - GRU (W_ih, W_hh) row-sharded over gates/H -> h_c [256] -> AllGather -> h
  - out_W row-sharded over V: each core computes 6250 logits + local sum-exp
    -> AllGather of the 8 partial sums -> local log-softmax normalization
  - biases folded into the matmuls as extra contraction rows (rank-1 update
    with a one-hot stationary operand), so they cost ~nothing

All matvecs y = A @ v run on the PE as out[1, n] += lhsT.T @ rhs with
lhsT = v k-slice [128, 1] (stationary) and rhs = A.T tile [128, n<=512]
(moving), with A.T pre-transposed on the host so every DMA is contiguous.
Vectors produced in free-layout [1, N] are converted to partition-layout
[128, N/128] via a PE transpose against an identity matrix.
"""

import numpy as np
import ml_dtypes

import concourse.bacc as bacc
import concourse.mybir as mybir
import concourse.tile as tile
from concourse import masks
from concourse.bass_utils import run_bass_kernel_spmd

# ---------------------------------------------------------------- constants
V, E, H, L = 50000, 300, 2048, 512
EH = E + H                      # 2348
KP = 2432                       # EH padded to 19*128 (incl. bias row at 2348)
NK = KP // 128                  # 19
NCORES = 8
VS = V // NCORES                # 6250 logits per core
HS = H // NCORES                # 256 hidden per core
GS = 3 * HS                     # 768 gate rows per core
F32 = mybir.dt.float32
BF16 = mybir.dt.bfloat16
FP8 = mybir.dt.float8e4

# dtype knobs per weight group (host cast + device compute dtype)
import os as _os
_KDT = _os.environ.get("KERNEL_DTYPES", "fbbb")  # attn, comb, gru, out
_DTMAP = {"f": F32, "b": BF16, "8": FP8}
DT_ATTN = _DTMAP[_KDT[0]]
DT_COMB = _DTMAP[_KDT[1]]
DT_GRU = _DTMAP[_KDT[2]]
DT_OUT = _DTMAP[_KDT[3]]

_NPDT = {F32: np.float32, BF16: ml_dtypes.bfloat16, FP8: ml_dtypes.float8_e4m3}

OUT_CHUNKS = [(j * 512, min(512, VS - j * 512)) for j in range((VS + 511) // 512)]


def _np(dt):
    return _NPDT[dt]


# ---------------------------------------------------------------- device IR
def build_nc():
    nc = bacc.Bacc(trn_type="TRN2", num_devices=NCORES, debug=False)

    def din(name, shape, dt):
        return nc.dram_tensor(name, shape, dt, kind="ExternalInput").ap()

    ident16 = din("ident16", [16, 16], F32)
    cat1_p = din("cat1_p", [128, NK], DT_ATTN)
    emb1_p = din("emb1_p", [128, 3], DT_COMB)
    h0g_p = din("h0g_p", [128, 16], DT_GRU)
    one_g = din("one_g", [128, 1], DT_GRU)
    one_o = din("one_o", [128, 1], DT_OUT)
    h0f = din("h0f", [1, HS], F32)
    attn_WT = din("attn_WT", [KP, L], DT_ATTN)
    enc = din("enc", [L, H], DT_ATTN)
    comb_WT = din("comb_WT", [KP, H], DT_COMB)
    wih_T = din("wih_T", [H, GS], DT_GRU)
    whh_T = din("whh_T", [H, GS], DT_GRU)
    gib = din("gib", [1, GS], DT_GRU)
    ghb = din("ghb", [1, GS], DT_GRU)
    outWT = din("outWT", [H, VS], DT_OUT)
    outb = din("outb", [1, VS], DT_OUT)

    logits_out = nc.dram_tensor("logits_out", [1, VS], F32, kind="ExternalOutput").ap()
    hidden_out = nc.dram_tensor("hidden_out", [1, H], F32, kind="ExternalOutput").ap()
    attnw_out = nc.dram_tensor("attnw_out", [1, L], F32, kind="ExternalOutput").ap()
    if _os.environ.get("KERNEL_DEBUG"):
        dbg_cat2 = nc.dram_tensor("dbg_cat2", [128, NK], DT_COMB,
                                  kind="ExternalOutput").ap()
        dbg_x = nc.dram_tensor("dbg_x", [1, H], F32, kind="ExternalOutput").ap()

    with tile.TileContext(nc) as tc:
        _body(nc, tc, locals())
    nc.compile()
    return nc


def _body(nc, tc, t):
    MM = nc.tensor.matmul
    import contextlib
    import itertools
    ctx = contextlib.ExitStack()
    # bulk DMAs round-robin over the two HWDGE queues (SP + ACT); tiny
    # latency-tolerant DMAs go via the GpSimd SWDGE queue.
    _bulk_cycle = itertools.cycle([nc.sync, nc.scalar])

    def BDMA(out, in_):
        next(_bulk_cycle).dma_start(out, in_)

    SDMA = nc.gpsimd.dma_start
    with ctx:
        const = ctx.enter_context(tc.tile_pool(name="const", bufs=1))
        # one shared streaming pool: every weight flows through [128, 2048]
        # slots so out_W prefetch can fill whatever the pre-phase isn't using
        strm = ctx.enter_context(tc.tile_pool(name="strm", bufs=33))
        fpool = ctx.enter_context(tc.tile_pool(name="fpool", bufs=2))
        ps_s = ctx.enter_context(tc.tile_pool(name="ps_s", bufs=3, space="PSUM"))
        ps_o = ctx.enter_context(tc.tile_pool(name="ps_o", bufs=5, space="PSUM"))
        dram = ctx.enter_context(tc.tile_pool(name="dram", bufs=1, space="DRAM"))

        DTSZ = {F32: 4, BF16: 2, FP8: 1}
        SLOTB = 4096  # bytes per partition per strm slot

        def stream_weight(name, src, kn, width, dt):
            """Stream src [kn*128, width] through [128, SLOTB] slots.
            Returns get(k, c0, w) -> AP of block k cols [c0, c0+w); the
            requested range must not straddle a tile boundary."""
            slotcols = SLOTB // DTSZ[dt]
            srcv = src.rearrange("(k p) n -> p k n", p=128)
            tiles = []
            if width <= slotcols:
                bpt = slotcols // width
                for ti in range((kn + bpt - 1) // bpt):
                    k0, k1 = ti * bpt, min((ti + 1) * bpt, kn)
                    w = strm.tile([128, slotcols], dt, tag="strm",
                                  name=f"{name}{ti}")
                    BDMA(w[:, 0:(k1 - k0) * width], srcv[:, k0:k1, :])
                    tiles.append(w)

                def get(k, c0, wd, _t=tiles, _bpt=bpt, _w=width):
                    return _t[k // _bpt][:, (k % _bpt) * _w + c0:
                                         (k % _bpt) * _w + c0 + wd]
            else:
                split = (width + slotcols - 1) // slotcols
                pc = width // split
                for k in range(kn):
                    for s in range(split):
                        w = strm.tile([128, slotcols], dt, tag="strm",
                                      name=f"{name}{k}_{s}")
                        BDMA(w[:, 0:pc], srcv[:, k, s * pc:(s + 1) * pc])
                        tiles.append(w)

                def get(k, c0, wd, _t=tiles, _split=split, _pc=pc):
                    ti = k * _split + c0 // _pc
                    return _t[ti][:, c0 % _pc:c0 % _pc + wd]
            return get

        # ---- constants / small inputs (SWDGE; ca1/ident first)
        ident = const.tile([16, 16], F32)
        SDMA(ident[:], t["ident16"])
        ca1 = const.tile([128, NK], DT_ATTN)
        SDMA(ca1[:], t["cat1_p"])
        emb1 = const.tile([128, 3], DT_COMB)
        SDMA(emb1[:], t["emb1_p"])
        h0g = const.tile([128, 16], DT_GRU)
        SDMA(h0g[:], t["h0g_p"])
        oneg = const.tile([128, 1], DT_GRU)
        SDMA(oneg[:], t["one_g"])
        oneo = const.tile([128, 1], DT_OUT)
        SDMA(oneo[:], t["one_o"])
        h0f_t = const.tile([1, HS], F32)
        SDMA(h0f_t[:], t["h0f"])
        gib_t = const.tile([1, GS], DT_GRU)
        SDMA(gib_t[:], t["gib"])
        ghb_t = const.tile([1, GS], DT_GRU)
        SDMA(ghb_t[:], t["ghb"])
        obt = const.tile([1, VS], DT_OUT)
        SDMA(obt[:], t["outb"])

        # ---- early sync point: a throwaway 4B AllGather absorbs core skew
        dsy_in = dram.tile([1, 1], F32)
        dsy_out = dram.tile([1, NCORES], F32)
        SDMA(dsy_in[:], h0f_t[:, 0:1])
        nc.gpsimd.collective_compute(
            "AllGather", mybir.AluOpType.bypass,
            replica_groups=[list(range(NCORES))],
            ins=[dsy_in.opt()], outs=[dsy_out.opt()])
        dsy_sb = const.tile([1, NCORES], F32)
        SDMA(dsy_sb[:], dsy_out[:])

        # ---- attention logits: al[1, 512] = cat1 @ attn_W.T (+attn_b via row 2348)
        awt = stream_weight("awt", t["attn_WT"], NK, L, DT_ATTN)
        psum_al = ps_s.tile([1, 512], F32, tag="pss")
        for k in range(NK):
            MM(psum_al[:], lhsT=ca1[:, k:k + 1], rhs=awt(k, 0, L),
               start=(k == 0), stop=(k == NK - 1))

        # ---- softmax over 512 on partition 0
        mx = const.tile([1, 1], F32)
        nc.vector.reduce_max(mx[:], psum_al[:], axis=mybir.AxisListType.X)
        negm = const.tile([1, 1], F32)
        nc.vector.tensor_scalar_mul(negm[:], mx[:], -1.0)
        e_sb = const.tile([1, 512], F32)
        s1 = const.tile([1, 1], F32)
        nc.scalar.activation(e_sb[:], psum_al[:], mybir.ActivationFunctionType.Exp,
                             bias=negm[:], scale=1.0, accum_out=s1[:])
        rs = const.tile([1, 1], F32)
        nc.vector.reciprocal(rs[:], s1[:])
        aw_sb = const.tile([1, 512], F32)
        nc.vector.tensor_scalar_mul(aw_sb[:], e_sb[:], rs[:])
        SDMA(t["attnw_out"], aw_sb[:])

        # ---- attn weights to partition layout [128, 4]
        aw4 = const.tile([4, 128], F32)
        for i in range(4):
            (nc.sync if i % 2 == 0 else nc.scalar).dma_start(
                aw4[i:i + 1, :], aw_sb[:, i * 128:(i + 1) * 128])
        ps_awp = ps_s.tile([128, 4], F32, tag="pss")
        nc.tensor.transpose(ps_awp[:], aw4[:], ident[0:4, 0:4])
        wp = const.tile([128, 4], DT_ATTN)
        nc.vector.tensor_copy(wp[:], ps_awp[:])

        # ---- attended context directly in partition layout [128, 16]
        # NOTE: accumulation groups within one PSUM tile must be contiguous
        # (start=True clears bank-wide), so loop j-outer / k-inner.
        encw = stream_weight("encw", t["enc"], 4, H, DT_ATTN)
        ps_aa = ps_s.tile([128, 16], F32, tag="pss")
        for j in range(16):
            for k in range(4):
                MM(ps_aa[:, j:j + 1], lhsT=encw(k, j * 128, 128),
                   rhs=wp[:, k:k + 1], start=(k == 0), stop=(k == 3))

        cat2 = const.tile([128, NK], DT_COMB)
        nc.vector.tensor_copy(cat2[:, 0:16], ps_aa[:])
        nc.scalar.copy(cat2[:, 16:19], emb1[:])
        if "dbg_cat2" in t:
            SDMA(t["dbg_cat2"], cat2[:])

        # ---- GRU weights early (so their DMAs queue ahead of out prefetch)
        whw = stream_weight("whw", t["whh_T"], 16, GS, DT_GRU)
        wiw = stream_weight("wiw", t["wih_T"], 16, GS, DT_GRU)

        # ---- attn_combine REPLICATED: x[1, 2048] = relu(cat2 @ comb_W.T + b)
        cwt = stream_weight("cwt", t["comb_WT"], NK, H, DT_COMB)
        ps_x = [ps_o.tile([1, 512], F32, tag="po", name=f"psx{j}")
                for j in range(4)]
        for k in range(NK):
            for j in range(4):
                MM(ps_x[j][:], lhsT=cat2[:, k:k + 1],
                   rhs=cwt(k, j * 512, 512),
                   start=(k == 0), stop=(k == NK - 1))
        x_sb = const.tile([1, H], F32)
        for j in range(4):
            nc.scalar.activation(x_sb[:, j * 512:(j + 1) * 512], ps_x[j][:],
                                 mybir.ActivationFunctionType.Relu)
        if "dbg_x" in t:
            SDMA(t["dbg_x"], x_sb[:])

        # ---- GRU gh half (independent of x)
        ps_gh_a = ps_s.tile([1, 512], F32, tag="pss")
        ps_gh_b = ps_s.tile([1, 512], F32, tag="pss")
        MM(ps_gh_a[:], lhsT=oneg[0:1, 0:1], rhs=ghb_t[:, 0:512],
           start=True, stop=False)
        MM(ps_gh_b[:, 0:HS], lhsT=oneg[0:1, 0:1], rhs=ghb_t[:, 512:768],
           start=True, stop=False)
        for k in range(16):
            MM(ps_gh_a[:], lhsT=h0g[:, k:k + 1], rhs=whw(k, 0, 512),
               start=False, stop=(k == 15))
            MM(ps_gh_b[:, 0:HS], lhsT=h0g[:, k:k + 1], rhs=whw(k, 512, HS),
               start=False, stop=(k == 15))
        gha = const.tile([1, 512], F32)
        nc.scalar.copy(gha[:], ps_gh_a[:])
        ghb_sb = const.tile([1, HS], F32)
        nc.scalar.copy(ghb_sb[:], ps_gh_b[:, 0:HS])

        # ---- x to partition layout (local; no collective)
        xf = const.tile([16, 128], F32)
        q3 = itertools.cycle([nc.sync, nc.scalar, nc.gpsimd])
        for i in range(16):
            next(q3).dma_start(xf[i:i + 1, :], x_sb[:, i * 128:(i + 1) * 128])
        ps_xp = ps_s.tile([128, 16], F32, tag="pss")
        nc.tensor.transpose(ps_xp[:], xf[:], ident[:])
        xp = const.tile([128, 16], DT_GRU)
        nc.vector.tensor_copy(xp[:], ps_xp[:])

        # ---- GRU gi half + gate math
        ps_gi_a = ps_s.tile([1, 512], F32, tag="pss")
        ps_gi_b = ps_s.tile([1, 512], F32, tag="pss")
        MM(ps_gi_a[:], lhsT=oneg[0:1, 0:1], rhs=gib_t[:, 0:512],
           start=True, stop=False)
        MM(ps_gi_b[:, 0:HS], lhsT=oneg[0:1, 0:1], rhs=gib_t[:, 512:768],
           start=True, stop=False)
        for k in range(16):
            MM(ps_gi_a[:], lhsT=xp[:, k:k + 1], rhs=wiw(k, 0, 512),
               start=False, stop=(k == 15))
            MM(ps_gi_b[:, 0:HS], lhsT=xp[:, k:k + 1], rhs=wiw(k, 512, HS),
               start=False, stop=(k == 15))

        rzpre = const.tile([1, 512], F32)
        nc.vector.tensor_add(rzpre[:], ps_gi_a[:], gha[:])
        rz = const.tile([1, 512], F32)
        nc.scalar.activation(rz[:], rzpre[:], mybir.ActivationFunctionType.Sigmoid)
        rhn = const.tile([1, HS], F32)
        nc.vector.tensor_mul(rhn[:], rz[:, 0:HS], ghb_sb[:])
        npre = const.tile([1, HS], F32)
        nc.vector.tensor_add(npre[:], ps_gi_b[:, 0:HS], rhn[:])
        n_sb = const.tile([1, HS], F32)
        nc.scalar.activation(n_sb[:], npre[:], mybir.ActivationFunctionType.Tanh)
        dd = const.tile([1, HS], F32)
        nc.vector.tensor_sub(dd[:], h0f_t[:], n_sb[:])
        zd = const.tile([1, HS], F32)
        nc.vector.tensor_mul(zd[:], rz[:, HS:2 * HS], dd[:])
        hnew = const.tile([1, HS], F32)
        nc.vector.tensor_add(hnew[:], n_sb[:], zd[:])

        # ---- AllGather h -> [2048]; emit hidden output; partition layout
        hin_d = dram.tile([1, HS], F32)
        hg_d = dram.tile([1, H], F32)
        nc.sync.dma_start(hin_d[:], hnew[:])
        nc.gpsimd.collective_compute(
            "AllGather", mybir.AluOpType.bypass,
            replica_groups=[list(range(NCORES))],
            ins=[hin_d.opt()], outs=[hg_d.opt()])
        SDMA(t["hidden_out"], hg_d[:])
        hf = const.tile([16, 128], F32)
        nc.sync.dma_start(hf[:], hg_d[:].rearrange("a (b c) -> (a b) c", c=128))
        ps_hp = ps_s.tile([128, 16], F32, tag="pss")
        nc.tensor.transpose(ps_hp[:], hf[:], ident[:])
        hp = const.tile([128, 16], DT_OUT)
        nc.vector.tensor_copy(hp[:], ps_hp[:])

        # ---- output projection row-shard: logits_c[6250] = h @ out_W_c.T + b_c
        ssum = const.tile([1, len(OUT_CHUNKS)], F32)
        lg_sb = const.tile([1, VS], F32)
        GRP = 4096 // DTSZ[DT_OUT]
        n_grp = (VS + GRP - 1) // GRP
        for g in range(n_grp):
            g0 = g * GRP
            gw = min(GRP, VS - g0)
            tiles_k = []
            for k in range(16):
                owt = strm.tile([128, GRP], DT_OUT, tag="strm",
                                name=f"owt{g}_{k}")
                BDMA(owt[:, 0:gw],
                     t["outWT"][k * 128:(k + 1) * 128, g0:g0 + gw])
                tiles_k.append(owt)
            for jj in range((gw + 511) // 512):
                j = g * (GRP // 512) + jj
                off, w = OUT_CHUNKS[j]
                ps = ps_o.tile([1, 512], F32, tag="po", name=f"po{j}")
                MM(ps[:, 0:w], lhsT=oneo[0:1, 0:1], rhs=obt[:, off:off + w],
                   start=True, stop=False)
                for k in range(16):
                    MM(ps[:, 0:w], lhsT=hp[:, k:k + 1],
                       rhs=tiles_k[k][:, jj * 512:jj * 512 + w],
                       start=False, stop=(k == 15))
                nc.scalar.copy(lg_sb[:, off:off + w], ps[:, 0:w])
                esc = fpool.tile([1, 512], F32, tag="esc", name=f"esc{j}")
                nc.scalar.activation(esc[:, 0:w], ps[:, 0:w],
                                     mybir.ActivationFunctionType.Exp,
                                     accum_out=ssum[:, j:j + 1])

        # ---- global log-sum-exp via AllGather of the 8 local sums
        sl = const.tile([1, 1], F32)
        nc.vector.reduce_sum(sl[:], ssum[:], axis=mybir.AxisListType.X)
        sin_d = dram.tile([1, 1], F32)
        sg_d = dram.tile([1, NCORES], F32)
        SDMA(sin_d[:], sl[:])
        nc.gpsimd.collective_compute(
            "AllGather", mybir.AluOpType.bypass,
            replica_groups=[list(range(NCORES))],
            ins=[sin_d.opt()], outs=[sg_d.opt()])
        s8 = const.tile([1, NCORES], F32)
        nc.sync.dma_start(s8[:], sg_d[:])
        st = const.tile([1, 1], F32)
        nc.vector.reduce_sum(st[:], s8[:], axis=mybir.AxisListType.X)
        logs = const.tile([1, 1], F32)
        nc.scalar.activation(logs[:], st[:], mybir.ActivationFunctionType.Ln)
        negls = const.tile([1, 1], F32)
        nc.vector.tensor_scalar_mul(negls[:], logs[:], -1.0)

        # ---- final normalize in place, split across DVE / ACT
        HALF = 3072
        nc.vector.tensor_scalar_add(lg_sb[:, 0:HALF], lg_sb[:, 0:HALF], negls[:])
        nc.scalar.activation(lg_sb[:, HALF:VS], lg_sb[:, HALF:VS],
                             mybir.ActivationFunctionType.Identity,
                             bias=negls[:], scale=1.0)
        nc.sync.dma_start(t["logits_out"][:, 0:HALF], lg_sb[:, 0:HALF])
        nc.scalar.dma_start(t["logits_out"][:, HALF:VS], lg_sb[:, HALF:VS])


# ---------------------------------------------------------------- host prep
def shard_inputs(input, hidden, encoder_outputs, emb, attn_W, attn_b,
                 comb_W, comb_b, W_ih, W_hh, b_ih, b_hh, out_W, out_b):
    """Build the 8 per-core input maps (numpy)."""
    idx = int(np.asarray(input).reshape(-1)[0])
    embedded = np.asarray(emb[idx], dtype=np.float32)          # [300]
    h0 = np.asarray(hidden, dtype=np.float32).reshape(H)       # [2048]
    attn_W = np.asarray(attn_W, dtype=np.float32)
    attn_b = np.asarray(attn_b, dtype=np.float32)
    comb_W = np.asarray(comb_W, dtype=np.float32)
    comb_b = np.asarray(comb_b, dtype=np.float32)
    W_ih = np.asarray(W_ih, dtype=np.float32)
    W_hh = np.asarray(W_hh, dtype=np.float32)
    b_ih = np.asarray(b_ih, dtype=np.float32)
    b_hh = np.asarray(b_hh, dtype=np.float32)
    out_W = np.asarray(out_W, dtype=np.float32)
    out_b = np.asarray(out_b, dtype=np.float32)
    enc = np.asarray(encoder_outputs, dtype=np.float32)

    # cat1 (reordered): [h0; embedded; 1.0; zeros] in partition layout
    cat1 = np.zeros(KP, dtype=np.float32)
    cat1[0:H] = h0
    cat1[H:H + E] = embedded
    cat1[EH] = 1.0
    cat1_p = np.ascontiguousarray(cat1.reshape(NK, 128).T, dtype=_np(DT_ATTN))
    emb1_p = np.ascontiguousarray(
        cat1[H:].reshape(3, 128).T, dtype=_np(DT_COMB))
    h0g_p = np.ascontiguousarray(h0.reshape(16, 128).T, dtype=_np(DT_GRU))
    one = np.zeros((128, 1), dtype=np.float32)
    one[0, 0] = 1.0

    # attn_W columns reordered to [h-part; e-part], bias row appended
    awt = np.zeros((KP, L), dtype=np.float32)
    awt[0:H] = attn_W[:, E:EH].T
    awt[H:EH] = attn_W[:, 0:E].T
    awt[EH] = attn_b
    awt = awt.astype(_np(DT_ATTN))

    enc_c = np.ascontiguousarray(enc, dtype=_np(DT_ATTN))

    cwt_full = np.zeros((KP, H), dtype=np.float32)
    cwt_full[0:H] = comb_W[:, E:EH].T
    cwt_full[H:EH] = comb_W[:, 0:E].T
    cwt_full[EH] = comb_b
    cwt_full = cwt_full.astype(_np(DT_COMB))

    per_core = []
    for c in range(NCORES):
        rows = slice(c * HS, (c + 1) * HS)
        grows = np.concatenate(
            [np.arange(g * H + c * HS, g * H + (c + 1) * HS) for g in range(3)])
        wih_t = np.ascontiguousarray(W_ih[grows].T, dtype=_np(DT_GRU))
        whh_t = np.ascontiguousarray(W_hh[grows].T, dtype=_np(DT_GRU))
        vrows = slice(c * VS, (c + 1) * VS)
        owt = np.ascontiguousarray(out_W[vrows].T, dtype=_np(DT_OUT))
        per_core.append({
            "ident16": np.eye(16, dtype=np.float32),
            "cat1_p": cat1_p,
            "emb1_p": emb1_p,
            "h0g_p": h0g_p,
            "one_g": one.astype(_np(DT_GRU)),
            "one_o": one.astype(_np(DT_OUT)),
            "h0f": h0[rows].reshape(1, HS).copy(),
            "attn_WT": awt,
            "enc": enc_c,
            "comb_WT": cwt_full,
            "wih_T": wih_t,
            "whh_T": whh_t,
            "gib": b_ih[grows].reshape(1, GS).astype(_np(DT_GRU)),
            "ghb": b_hh[grows].reshape(1, GS).astype(_np(DT_GRU)),
            "outWT": owt,
            "outb": out_b[vrows].reshape(1, VS).astype(_np(DT_OUT)),
        })
    return per_core


# ---------------------------------------------------------------- entry
_CACHED_NC = None
LAST_RESULT = None


def kernel(**inputs):
    global _CACHED_NC, LAST_RESULT
    try:
        import axon_profile_shim
        axon_profile_shim.install()
    except Exception:
        pass
    if _CACHED_NC is None:
        _CACHED_NC = build_nc()
    in_maps = shard_inputs(**inputs)
    trace = bool(int(__import__("os").environ.get("KERNEL_TRACE", "0")))
    res = run_bass_kernel_spmd(
        _CACHED_NC, in_maps, core_ids=list(range(NCORES)), trace=trace)
    LAST_RESULT = res
    logits = np.concatenate(
        [res.results[c]["logits_out"] for c in range(NCORES)], axis=1)
    hidden_new = res.results[0]["hidden_out"].reshape(1, 1, H)
    attn_weights = res.results[0]["attnw_out"].reshape(1, L)
    return (logits.astype(np.float32),
            hidden_new.astype(np.float32),
            attn_weights.astype(np.float32))


# revision 27
# speedup vs baseline: 1.1757x; 1.1757x over previous
"""Trainium2 Bass kernel for single-step AttnDecoderRNN (batch=1 decode).

Strategy (8-way tensor parallel, one NEFF, SPMD):
  - embedding gather happens on host (only the one needed row is shipped)
  - attention (attn_W, encoder_outputs) replicated: every core computes the
    full 512 attn weights and the full attended context (cheap: ~9MB)
  - attn_combine row-sharded over H  -> x_c [256]     -> AllGather -> x [2048]
  - GRU (W_ih, W_hh) row-sharded over gates/H -> h_c [256] -> AllGather -> h
  - out_W row-sharded over V: each core computes 6250 logits + local sum-exp
    -> AllGather of the 8 partial sums -> local log-softmax normalization
  - biases folded into the matmuls as extra contraction rows (rank-1 update
    with a one-hot stationary operand), so they cost ~nothing

All matvecs y = A @ v run on the PE as out[1, n] += lhsT.T @ rhs with
lhsT = v k-slice [128, 1] (stationary) and rhs = A.T tile [128, n<=512]
(moving), with A.T pre-transposed on the host so every DMA is contiguous.
Vectors produced in free-layout [1, N] are converted to partition-layout
[128, N/128] via a PE transpose against an identity matrix.
"""

import numpy as np
import ml_dtypes

import concourse.bacc as bacc
import concourse.mybir as mybir
import concourse.tile as tile
from concourse import masks
from concourse.bass_utils import run_bass_kernel_spmd

# ---------------------------------------------------------------- constants
V, E, H, L = 50000, 300, 2048, 512
EH = E + H                      # 2348
KP = 2432                       # EH padded to 19*128 (incl. bias row at 2348)
NK = KP // 128                  # 19
NCORES = 8
VS = V // NCORES                # 6250 logits per core
HS = H // NCORES                # 256 hidden per core
GS = 3 * HS                     # 768 gate rows per core
F32 = mybir.dt.float32
BF16 = mybir.dt.bfloat16
FP8 = mybir.dt.float8e4

# dtype knobs per weight group (host cast + device compute dtype)
import os as _os
_KDT = _os.environ.get("KERNEL_DTYPES", "fbbb")  # attn, comb, gru, out
_DTMAP = {"f": F32, "b": BF16, "8": FP8}
DT_ATTN = _DTMAP[_KDT[0]]
DT_COMB = _DTMAP[_KDT[1]]
DT_GRU = _DTMAP[_KDT[2]]
DT_OUT = _DTMAP[_KDT[3]]
DT_ENC = _DTMAP[_KDT[4]] if len(_KDT) > 4 else BF16
# fp8 weights are host-scaled into e4m3's normal range; the inverse scale
# is folded into the psum-consuming activation
SC_COMB = 64.0 if DT_COMB == FP8 else 1.0
SC_OUT = 64.0 if DT_OUT == FP8 else 1.0

_NPDT = {F32: np.float32, BF16: ml_dtypes.bfloat16, FP8: ml_dtypes.float8_e4m3}

OUT_CHUNKS = [(j * 512, min(512, VS - j * 512)) for j in range((VS + 511) // 512)]


def _np(dt):
    return _NPDT[dt]


# ---------------------------------------------------------------- device IR
def build_nc():
    nc = bacc.Bacc(trn_type="TRN2", num_devices=NCORES, debug=False)

    def din(name, shape, dt):
        return nc.dram_tensor(name, shape, dt, kind="ExternalInput").ap()

    ident16 = din("ident16", [16, 16], F32)
    cat1_p = din("cat1_p", [128, NK], DT_ATTN)
    emb1_p = din("emb1_p", [128, 3], DT_COMB)
    h0g_p = din("h0g_p", [128, 16], DT_GRU)
    one_g = din("one_g", [128, 1], DT_GRU)
    one_o = din("one_o", [128, 1], DT_OUT)
    h0f = din("h0f", [1, HS], F32)
    attn_WT = din("attn_WT", [KP, L], DT_ATTN)
    enc = din("enc", [L, H], DT_ENC)
    comb_WT = din("comb_WT", [KP, H], DT_COMB)
    wih_T = din("wih_T", [H, GS], DT_GRU)
    whh_T = din("whh_T", [H, GS], DT_GRU)
    gib = din("gib", [1, GS], DT_GRU)
    ghb = din("ghb", [1, GS], DT_GRU)
    outWT = din("outWT", [H, VS], DT_OUT)
    outb = din("outb", [1, VS], DT_OUT)

    logits_out = nc.dram_tensor("logits_out", [1, VS], F32, kind="ExternalOutput").ap()
    hidden_out = nc.dram_tensor("hidden_out", [1, H], F32, kind="ExternalOutput").ap()
    attnw_out = nc.dram_tensor("attnw_out", [1, L], F32, kind="ExternalOutput").ap()
    if _os.environ.get("KERNEL_DEBUG"):
        dbg_cat2 = nc.dram_tensor("dbg_cat2", [128, NK], DT_COMB,
                                  kind="ExternalOutput").ap()
        dbg_x = nc.dram_tensor("dbg_x", [1, H], F32, kind="ExternalOutput").ap()

    with tile.TileContext(nc) as tc:
        _body(nc, tc, locals())
    nc.compile()
    return nc


def _body(nc, tc, t):
    MM = nc.tensor.matmul
    import contextlib
    import itertools
    ctx = contextlib.ExitStack()
    # bulk DMAs round-robin over the two HWDGE queues (SP + ACT); tiny
    # latency-tolerant DMAs go via the GpSimd SWDGE queue.
    _bulk_cycle = itertools.cycle([nc.sync, nc.scalar])

    def BDMA(out, in_):
        next(_bulk_cycle).dma_start(out, in_)

    SDMA = nc.gpsimd.dma_start
    with ctx:
        const = ctx.enter_context(tc.tile_pool(name="const", bufs=1))
        # one shared streaming pool: every weight flows through [128, 2048]
        # slots so out_W prefetch can fill whatever the pre-phase isn't using
        strm = ctx.enter_context(tc.tile_pool(name="strm", bufs=33))
        fpool = ctx.enter_context(tc.tile_pool(name="fpool", bufs=2))
        ps_s = ctx.enter_context(tc.tile_pool(name="ps_s", bufs=3, space="PSUM"))
        ps_o = ctx.enter_context(tc.tile_pool(name="ps_o", bufs=5, space="PSUM"))
        dram = ctx.enter_context(tc.tile_pool(name="dram", bufs=1, space="DRAM"))

        DTSZ = {F32: 4, BF16: 2, FP8: 1}
        SLOTB = 4096  # bytes per partition per strm slot

        def stream_weight(name, src, kn, width, dt):
            """Stream src [kn*128, width] through [128, SLOTB] slots.
            Returns get(k, c0, w) -> AP of block k cols [c0, c0+w); the
            requested range must not straddle a tile boundary."""
            slotcols = SLOTB // DTSZ[dt]
            srcv = src.rearrange("(k p) n -> p k n", p=128)
            tiles = []
            if width <= slotcols:
                bpt = slotcols // width
                for ti in range((kn + bpt - 1) // bpt):
                    k0, k1 = ti * bpt, min((ti + 1) * bpt, kn)
                    w = strm.tile([128, slotcols], dt, tag="strm",
                                  name=f"{name}{ti}")
                    BDMA(w[:, 0:(k1 - k0) * width], srcv[:, k0:k1, :])
                    tiles.append(w)

                def get(k, c0, wd, _t=tiles, _bpt=bpt, _w=width):
                    return _t[k // _bpt][:, (k % _bpt) * _w + c0:
                                         (k % _bpt) * _w + c0 + wd]
            else:
                split = (width + slotcols - 1) // slotcols
                pc = width // split
                for k in range(kn):
                    for s in range(split):
                        w = strm.tile([128, slotcols], dt, tag="strm",
                                      name=f"{name}{k}_{s}")
                        BDMA(w[:, 0:pc], srcv[:, k, s * pc:(s + 1) * pc])
                        tiles.append(w)

                def get(k, c0, wd, _t=tiles, _split=split, _pc=pc):
                    ti = k * _split + c0 // _pc
                    return _t[ti][:, c0 % _pc:c0 % _pc + wd]
            return get

        # ---- constants / small inputs (SWDGE; ca1/ident first)
        ident = const.tile([16, 16], F32)
        SDMA(ident[:], t["ident16"])
        ca1 = const.tile([128, NK], DT_ATTN)
        SDMA(ca1[:], t["cat1_p"])
        emb1 = const.tile([128, 3], DT_COMB)
        SDMA(emb1[:], t["emb1_p"])
        h0g = const.tile([128, 16], DT_GRU)
        SDMA(h0g[:], t["h0g_p"])
        oneg = const.tile([128, 1], DT_GRU)
        SDMA(oneg[:], t["one_g"])
        oneo = const.tile([128, 1], DT_OUT)
        SDMA(oneo[:], t["one_o"])
        h0f_t = const.tile([1, HS], F32)
        SDMA(h0f_t[:], t["h0f"])
        gib_t = const.tile([1, GS], DT_GRU)
        SDMA(gib_t[:], t["gib"])
        ghb_t = const.tile([1, GS], DT_GRU)
        SDMA(ghb_t[:], t["ghb"])
        obt = const.tile([1, VS], DT_OUT)
        SDMA(obt[:], t["outb"])

        # ---- early sync point: a throwaway 4B AllGather absorbs core skew
        dsy_in = dram.tile([1, 1], F32)
        dsy_out = dram.tile([1, NCORES], F32)
        SDMA(dsy_in[:], h0f_t[:, 0:1])
        nc.gpsimd.collective_compute(
            "AllGather", mybir.AluOpType.bypass,
            replica_groups=[list(range(NCORES))],
            ins=[dsy_in.opt()], outs=[dsy_out.opt()])
        dsy_sb = const.tile([1, NCORES], F32)
        SDMA(dsy_sb[:], dsy_out[:])

        # ---- attention logits: al[1, 512] = cat1 @ attn_W.T (+attn_b via row 2348)
        awt = stream_weight("awt", t["attn_WT"], NK, L, DT_ATTN)
        psum_al = ps_s.tile([1, 512], F32, tag="pss")
        for k in range(NK):
            MM(psum_al[:], lhsT=ca1[:, k:k + 1], rhs=awt(k, 0, L),
               start=(k == 0), stop=(k == NK - 1))

        # ---- softmax over 512 on partition 0
        mx = const.tile([1, 1], F32)
        nc.vector.reduce_max(mx[:], psum_al[:], axis=mybir.AxisListType.X)
        negm = const.tile([1, 1], F32)
        nc.vector.tensor_scalar_mul(negm[:], mx[:], -1.0)
        e_sb = const.tile([1, 512], F32)
        s1 = const.tile([1, 1], F32)
        nc.scalar.activation(e_sb[:], psum_al[:], mybir.ActivationFunctionType.Exp,
                             bias=negm[:], scale=1.0, accum_out=s1[:])
        rs = const.tile([1, 1], F32)
        nc.vector.reciprocal(rs[:], s1[:])
        aw_sb = const.tile([1, 512], F32)
        nc.vector.tensor_scalar_mul(aw_sb[:], e_sb[:], rs[:])
        SDMA(t["attnw_out"], aw_sb[:])

        # ---- attn weights to partition layout [128, 4]
        aw4 = const.tile([4, 128], F32)
        for i in range(4):
            (nc.sync if i % 2 == 0 else nc.scalar).dma_start(
                aw4[i:i + 1, :], aw_sb[:, i * 128:(i + 1) * 128])
        ps_awp = ps_s.tile([128, 4], F32, tag="pss")
        nc.tensor.transpose(ps_awp[:], aw4[:], ident[0:4, 0:4])
        wp = const.tile([128, 4], DT_ENC)
        nc.vector.tensor_copy(wp[:], ps_awp[:])

        # ---- attended context directly in partition layout [128, 16]
        # NOTE: accumulation groups within one PSUM tile must be contiguous
        # (start=True clears bank-wide), so loop j-outer / k-inner.
        encw = stream_weight("encw", t["enc"], 4, H, DT_ENC)
        ps_aa = ps_s.tile([128, 16], F32, tag="pss")
        for j in range(16):
            for k in range(4):
                MM(ps_aa[:, j:j + 1], lhsT=encw(k, j * 128, 128),
                   rhs=wp[:, k:k + 1], start=(k == 0), stop=(k == 3))

        cat2 = const.tile([128, NK], DT_COMB)
        nc.vector.tensor_copy(cat2[:, 0:16], ps_aa[:])
        nc.scalar.copy(cat2[:, 16:19], emb1[:])
        if "dbg_cat2" in t:
            SDMA(t["dbg_cat2"], cat2[:])

        # ---- GRU weights early (so their DMAs queue ahead of out prefetch)
        whw = stream_weight("whw", t["whh_T"], 16, GS, DT_GRU)
        wiw = stream_weight("wiw", t["wih_T"], 16, GS, DT_GRU)

        # ---- attn_combine REPLICATED: x[1, 2048] = relu(cat2 @ comb_W.T + b)
        cwt = stream_weight("cwt", t["comb_WT"], NK, H, DT_COMB)
        ps_x = [ps_o.tile([1, 512], F32, tag="po", name=f"psx{j}")
                for j in range(4)]
        for k in range(NK):
            for j in range(4):
                MM(ps_x[j][:], lhsT=cat2[:, k:k + 1],
                   rhs=cwt(k, j * 512, 512),
                   start=(k == 0), stop=(k == NK - 1))
        x_sb = const.tile([1, H], F32)
        for j in range(4):
            nc.scalar.activation(x_sb[:, j * 512:(j + 1) * 512], ps_x[j][:],
                                 mybir.ActivationFunctionType.Relu,
                                 scale=1.0 / SC_COMB)
        if "dbg_x" in t:
            SDMA(t["dbg_x"], x_sb[:])

        # ---- GRU gh half (independent of x)
        ps_gh_a = ps_s.tile([1, 512], F32, tag="pss")
        ps_gh_b = ps_s.tile([1, 512], F32, tag="pss")
        MM(ps_gh_a[:], lhsT=oneg[0:1, 0:1], rhs=ghb_t[:, 0:512],
           start=True, stop=False)
        MM(ps_gh_b[:, 0:HS], lhsT=oneg[0:1, 0:1], rhs=ghb_t[:, 512:768],
           start=True, stop=False)
        for k in range(16):
            MM(ps_gh_a[:], lhsT=h0g[:, k:k + 1], rhs=whw(k, 0, 512),
               start=False, stop=(k == 15))
            MM(ps_gh_b[:, 0:HS], lhsT=h0g[:, k:k + 1], rhs=whw(k, 512, HS),
               start=False, stop=(k == 15))
        gha = const.tile([1, 512], F32)
        nc.scalar.copy(gha[:], ps_gh_a[:])
        ghb_sb = const.tile([1, HS], F32)
        nc.scalar.copy(ghb_sb[:], ps_gh_b[:, 0:HS])

        # ---- x to partition layout (local; no collective)
        xf = const.tile([16, 128], F32)
        q3 = itertools.cycle([nc.sync, nc.scalar, nc.gpsimd])
        for i in range(16):
            next(q3).dma_start(xf[i:i + 1, :], x_sb[:, i * 128:(i + 1) * 128])
        ps_xp = ps_s.tile([128, 16], F32, tag="pss")
        nc.tensor.transpose(ps_xp[:], xf[:], ident[:])
        xp = const.tile([128, 16], DT_GRU)
        nc.vector.tensor_copy(xp[:], ps_xp[:])

        # ---- GRU gi half + gate math
        ps_gi_a = ps_s.tile([1, 512], F32, tag="pss")
        ps_gi_b = ps_s.tile([1, 512], F32, tag="pss")
        MM(ps_gi_a[:], lhsT=oneg[0:1, 0:1], rhs=gib_t[:, 0:512],
           start=True, stop=False)
        MM(ps_gi_b[:, 0:HS], lhsT=oneg[0:1, 0:1], rhs=gib_t[:, 512:768],
           start=True, stop=False)
        for k in range(16):
            MM(ps_gi_a[:], lhsT=xp[:, k:k + 1], rhs=wiw(k, 0, 512),
               start=False, stop=(k == 15))
            MM(ps_gi_b[:, 0:HS], lhsT=xp[:, k:k + 1], rhs=wiw(k, 512, HS),
               start=False, stop=(k == 15))

        rzpre = const.tile([1, 512], F32)
        nc.vector.tensor_add(rzpre[:], ps_gi_a[:], gha[:])
        rz = const.tile([1, 512], F32)
        nc.scalar.activation(rz[:], rzpre[:], mybir.ActivationFunctionType.Sigmoid)
        rhn = const.tile([1, HS], F32)
        nc.vector.tensor_mul(rhn[:], rz[:, 0:HS], ghb_sb[:])
        npre = const.tile([1, HS], F32)
        nc.vector.tensor_add(npre[:], ps_gi_b[:, 0:HS], rhn[:])
        n_sb = const.tile([1, HS], F32)
        nc.scalar.activation(n_sb[:], npre[:], mybir.ActivationFunctionType.Tanh)
        dd = const.tile([1, HS], F32)
        nc.vector.tensor_sub(dd[:], h0f_t[:], n_sb[:])
        zd = const.tile([1, HS], F32)
        nc.vector.tensor_mul(zd[:], rz[:, HS:2 * HS], dd[:])
        hnew = const.tile([1, HS], F32)
        nc.vector.tensor_add(hnew[:], n_sb[:], zd[:])

        # ---- AllGather h -> [2048]; emit hidden output; partition layout
        hin_d = dram.tile([1, HS], F32)
        hg_d = dram.tile([1, H], F32)
        nc.sync.dma_start(hin_d[:], hnew[:])
        nc.gpsimd.collective_compute(
            "AllGather", mybir.AluOpType.bypass,
            replica_groups=[list(range(NCORES))],
            ins=[hin_d.opt()], outs=[hg_d.opt()])
        SDMA(t["hidden_out"], hg_d[:])
        hf = const.tile([16, 128], F32)
        nc.sync.dma_start(hf[:], hg_d[:].rearrange("a (b c) -> (a b) c", c=128))
        ps_hp = ps_s.tile([128, 16], F32, tag="pss")
        nc.tensor.transpose(ps_hp[:], hf[:], ident[:])
        hp = const.tile([128, 16], DT_OUT)
        nc.vector.tensor_copy(hp[:], ps_hp[:])

        # ---- output projection row-shard: logits_c[6250] = h @ out_W_c.T + b_c
        ssum = const.tile([1, len(OUT_CHUNKS)], F32)
        lg_sb = const.tile([1, VS], F32)
        GRP = 4096 // DTSZ[DT_OUT]
        n_grp = (VS + GRP - 1) // GRP
        for g in range(n_grp):
            g0 = g * GRP
            gw = min(GRP, VS - g0)
            tiles_k = []
            for k in range(16):
                owt = strm.tile([128, GRP], DT_OUT, tag="strm",
                                name=f"owt{g}_{k}")
                BDMA(owt[:, 0:gw],
                     t["outWT"][k * 128:(k + 1) * 128, g0:g0 + gw])
                tiles_k.append(owt)
            for jj in range((gw + 511) // 512):
                j = g * (GRP // 512) + jj
                off, w = OUT_CHUNKS[j]
                ps = ps_o.tile([1, 512], F32, tag="po", name=f"po{j}")
                MM(ps[:, 0:w], lhsT=oneo[0:1, 0:1], rhs=obt[:, off:off + w],
                   start=True, stop=False)
                for k in range(16):
                    MM(ps[:, 0:w], lhsT=hp[:, k:k + 1],
                       rhs=tiles_k[k][:, jj * 512:jj * 512 + w],
                       start=False, stop=(k == 15))
                nc.scalar.mul(lg_sb[:, off:off + w], ps[:, 0:w], 1.0 / SC_OUT)
                esc = fpool.tile([1, 512], F32, tag="esc", name=f"esc{j}")
                nc.scalar.activation(esc[:, 0:w], ps[:, 0:w],
                                     mybir.ActivationFunctionType.Exp,
                                     scale=1.0 / SC_OUT,
                                     accum_out=ssum[:, j:j + 1])

        # ---- global log-sum-exp via AllGather of the 8 local sums
        sl = const.tile([1, 1], F32)
        nc.vector.reduce_sum(sl[:], ssum[:], axis=mybir.AxisListType.X)
        sin_d = dram.tile([1, 1], F32)
        sg_d = dram.tile([1, NCORES], F32)
        SDMA(sin_d[:], sl[:])
        nc.gpsimd.collective_compute(
            "AllGather", mybir.AluOpType.bypass,
            replica_groups=[list(range(NCORES))],
            ins=[sin_d.opt()], outs=[sg_d.opt()])
        s8 = const.tile([1, NCORES], F32)
        nc.sync.dma_start(s8[:], sg_d[:])
        st = const.tile([1, 1], F32)
        nc.vector.reduce_sum(st[:], s8[:], axis=mybir.AxisListType.X)
        logs = const.tile([1, 1], F32)
        nc.scalar.activation(logs[:], st[:], mybir.ActivationFunctionType.Ln)
        negls = const.tile([1, 1], F32)
        nc.vector.tensor_scalar_mul(negls[:], logs[:], -1.0)

        # ---- final normalize in place, split across DVE / ACT
        HALF = 3072
        nc.vector.tensor_scalar_add(lg_sb[:, 0:HALF], lg_sb[:, 0:HALF], negls[:])
        nc.scalar.activation(lg_sb[:, HALF:VS], lg_sb[:, HALF:VS],
                             mybir.ActivationFunctionType.Identity,
                             bias=negls[:], scale=1.0)
        nc.sync.dma_start(t["logits_out"][:, 0:HALF], lg_sb[:, 0:HALF])
        nc.scalar.dma_start(t["logits_out"][:, HALF:VS], lg_sb[:, HALF:VS])


# ---------------------------------------------------------------- host prep
def shard_inputs(input, hidden, encoder_outputs, emb, attn_W, attn_b,
                 comb_W, comb_b, W_ih, W_hh, b_ih, b_hh, out_W, out_b):
    """Build the 8 per-core input maps (numpy)."""
    idx = int(np.asarray(input).reshape(-1)[0])
    embedded = np.asarray(emb[idx], dtype=np.float32)          # [300]
    h0 = np.asarray(hidden, dtype=np.float32).reshape(H)       # [2048]
    attn_W = np.asarray(attn_W, dtype=np.float32)
    attn_b = np.asarray(attn_b, dtype=np.float32)
    comb_W = np.asarray(comb_W, dtype=np.float32)
    comb_b = np.asarray(comb_b, dtype=np.float32)
    W_ih = np.asarray(W_ih, dtype=np.float32)
    W_hh = np.asarray(W_hh, dtype=np.float32)
    b_ih = np.asarray(b_ih, dtype=np.float32)
    b_hh = np.asarray(b_hh, dtype=np.float32)
    out_W = np.asarray(out_W, dtype=np.float32)
    out_b = np.asarray(out_b, dtype=np.float32)
    enc = np.asarray(encoder_outputs, dtype=np.float32)

    # cat1 (reordered): [h0; embedded; 1.0; zeros] in partition layout
    cat1 = np.zeros(KP, dtype=np.float32)
    cat1[0:H] = h0
    cat1[H:H + E] = embedded
    cat1[EH] = 1.0
    cat1_p = np.ascontiguousarray(cat1.reshape(NK, 128).T, dtype=_np(DT_ATTN))
    emb1_p = np.ascontiguousarray(
        cat1[H:].reshape(3, 128).T, dtype=_np(DT_COMB))
    h0g_p = np.ascontiguousarray(h0.reshape(16, 128).T, dtype=_np(DT_GRU))
    one = np.zeros((128, 1), dtype=np.float32)
    one[0, 0] = 1.0

    # attn_W columns reordered to [h-part; e-part], bias row appended
    awt = np.zeros((KP, L), dtype=np.float32)
    awt[0:H] = attn_W[:, E:EH].T
    awt[H:EH] = attn_W[:, 0:E].T
    awt[EH] = attn_b
    awt = awt.astype(_np(DT_ATTN))

    enc_c = np.ascontiguousarray(enc, dtype=_np(DT_ENC))

    cwt_full = np.zeros((KP, H), dtype=np.float32)
    cwt_full[0:H] = comb_W[:, E:EH].T
    cwt_full[H:EH] = comb_W[:, 0:E].T
    cwt_full[EH] = comb_b
    cwt_full = (cwt_full * SC_COMB).astype(_np(DT_COMB))

    per_core = []
    for c in range(NCORES):
        rows = slice(c * HS, (c + 1) * HS)
        grows = np.concatenate(
            [np.arange(g * H + c * HS, g * H + (c + 1) * HS) for g in range(3)])
        wih_t = np.ascontiguousarray(W_ih[grows].T, dtype=_np(DT_GRU))
        whh_t = np.ascontiguousarray(W_hh[grows].T, dtype=_np(DT_GRU))
        vrows = slice(c * VS, (c + 1) * VS)
        owt = np.ascontiguousarray(
            out_W[vrows].T * SC_OUT, dtype=_np(DT_OUT))
        per_core.append({
            "ident16": np.eye(16, dtype=np.float32),
            "cat1_p": cat1_p,
            "emb1_p": emb1_p,
            "h0g_p": h0g_p,
            "one_g": one.astype(_np(DT_GRU)),
            "one_o": one.astype(_np(DT_OUT)),
            "h0f": h0[rows].reshape(1, HS).copy(),
            "attn_WT": awt,
            "enc": enc_c,
            "comb_WT": cwt_full,
            "wih_T": wih_t,
            "whh_T": whh_t,
            "gib": b_ih[grows].reshape(1, GS).astype(_np(DT_GRU)),
            "ghb": b_hh[grows].reshape(1, GS).astype(_np(DT_GRU)),
            "outWT": owt,
            "outb": (out_b[vrows].reshape(1, VS) * SC_OUT).astype(_np(DT_OUT)),
        })
    return per_core


# ---------------------------------------------------------------- entry
_CACHED_NC = None
LAST_RESULT = None


def kernel(**inputs):
    global _CACHED_NC, LAST_RESULT
    try:
        import axon_profile_shim
        axon_profile_shim.install()
    except Exception:
        pass
    if _CACHED_NC is None:
        _CACHED_NC = build_nc()
    in_maps = shard_inputs(**inputs)
    trace = bool(int(__import__("os").environ.get("KERNEL_TRACE", "0")))
    res = run_bass_kernel_spmd(
        _CACHED_NC, in_maps, core_ids=list(range(NCORES)), trace=trace)
    LAST_RESULT = res
    logits = np.concatenate(
        [res.results[c]["logits_out"] for c in range(NCORES)], axis=1)
    hidden_new = res.results[0]["hidden_out"].reshape(1, 1, H)
    attn_weights = res.results[0]["attnw_out"].reshape(1, L)
    return (logits.astype(np.float32),
            hidden_new.astype(np.float32),
            attn_weights.astype(np.float32))
